# revision 33
# baseline (speedup 1.0000x reference)
"""PASA group-softmax downsample kernel for 8 Trainium2 NeuronCores.

Reference computation (per reference.py):
  x (2, 64, 32, 32, 32) f32
  xp = reflect-pad x by 1 on d/h/w
  sigma = conv3d(xp, conv_w (54, 64, 3,3,3), stride 1, valid)   -> (2, 54, 32,32,32)
  sigma = batchnorm(sigma, batch stats over (n,d,h,w), gamma, beta)
  sigma = softmax(sigma, axis=1)
  out[n,g,cc,o] = sum_p patches[n,g,cc,p,o] * sigma[n,g*27+p,o]  (g=2 groups of 32 ch)
  return out[:, :, ::2, ::2, ::2]                                -> (2, 64, 16, 16, 16)

Sharding: 8 shards = (batch n in {0,1}) x (4 depth chunks of 8 planes).
Each core gets a padded depth slab with halo (10 planes of the padded
volume).  Two SPMD launches with a host relay for the BN-stat reduction
and softmax (a device AllReduce costs launch-skew, measured ~80us).

Launch A (per core):
  * fp8 (e4m3) full-volume conv in DoubleRow perf mode (K_eff=256: 2
    taps via a shifted copy in partitions 64..127, 2 more via the
    k-tile pair dim) -- 8 matmul units instead of 15, 2x column rate.
    Result sigma_q is shipped to the host as fp16 (54, 8704) and used
    ONLY for BN statistics (fp8 noise averages out over 65536
    positions; residual bias removed on host, see below).
  * fp16 strided conv at the 1024 strided output positions only (14
    units over a parity-decomposed slab layout) -> ssub16 (54, 1024)
    f32: the accurate softmax logits.

Host: global BN stats from sigma_q; the fp8-induced bias of mean/var is
  estimated from the strided positions where both fp8 and fp16 conv
  values exist (sampling error cancels in the difference) and
  subtracted.  Then a = gamma*rstd, b = beta - mean*a, softmax of
  a*ssub16+b over the 54 channels -> en (attention), shipped fp16.

Launch B (per core): two 128-partition passes (chunk pair per pass:
  strided plane k in partitions 0..63, k+1 in 64..127).  The attention
  is shipped compact (en8, ~0.9MB with 3 replicas) and replicated
  across the 32 channels of each group ON-CHIP by the otherwise-idle
  tensor engine (selector matmuls, K=4) + ACT fp32->fp16 converts --
  a DMA partition-broadcast of the full 3.5MB replication was the
  launch-B bottleneck.  Per pass: 9 DVE multiplies (di-fused windows,
  768 elems/partition each, fp16 2x mode) against the parity-
  decomposed fp16 slab, then a 5-op halving tree over the 27 tap
  blocks; outputs ship fp16, DMA out (64, 4, 16, 16).
"""

import sys

sys.path.insert(0, "/opt/trn_rl_repo")

import ml_dtypes
import numpy as np

import concourse.bacc as bacc
import concourse.mybir as mybir
from concourse import bass_utils, tile

N_CORES = 8
K = 3
GROUP = 2
STRIDE = 2
EPS = 1e-5

N, C, D, H, W = 2, 64, 32, 32, 32
COUT = GROUP * K * K * K  # 54
PD, PH, PW = D + 2, H + 2, W + 2  # 34, 34, 34
ZP = 10  # padded slab planes per core
PLANE = PH * PW  # 1156
XLEN = ZP * PLANE  # 11560
DL = 8  # local output depth extent
SPAN = H * PW  # 1088 flat conv cols per plane (h<32, w<34; w>=32 junk)
SPOS = (DL // 2) * (H // 2) * (W // 2)  # 1024 strided positions per core
M_TOTAL = float(N * D * H * W)  # 65536 positions for BN stats

F32 = mybir.dt.float32
F16 = mybir.dt.float16
F8 = mybir.dt.float8e4
E4 = ml_dtypes.float8_e4m3

XB8A = XLEN + 8  # fp8 slab + (+34)-shifted copy in upper partitions
XB8B = XLEN + 80  # fp8 slab + (+1156)-shifted copy; U8 k-tile overruns past end
CSPAN = DL * SPAN  # 8704 output cols (32 rows x 34; junk cols host-masked)
PE_ = 289  # 17*17 parity plane
XPE_LEN = 9 * 2 * PE_  # 5202
XPO_LEN = 9 * PE_  # 2601
XPZ_LEN = 5 * PE_  # 1445

# fp8 stats-conv units: (x tensor, rhs base offset fn of d, k-tile stride)
# x8a: upper partitions = slab shifted +34 (hj pair); x8b: +1156 (di pair).
# Weight taps per unit/k-tile/half are packed host-side in _pack_w8.
U8A = [
    ("a", lambda d: (d + 0) * PLANE, 1),
    ("a", lambda d: (d + 1) * PLANE, 1),
    ("a", lambda d: (d + 2) * PLANE, 1),
    ("a", lambda d: d * PLANE + 2, PLANE),
    ("a", lambda d: (d + 2) * PLANE + 2, 1),
    ("b", lambda d: d * PLANE + 2 * PW, 1),
    ("b", lambda d: (d + 2) * PLANE + 2 * PW, 1),
    ("b", lambda d: d * PLANE + 2 * PW + 2, 2 * PLANE),
]
NU8 = len(U8A)  # 8

# fp16 strided-conv units: (tensor, base offset, plane-pair stride, K)
# xpe: lower=x-even parity, upper=x-odd (wl 0/1 pair), z-major blocks of 578
# xpo: lower=y-even, upper=y-odd (hj 0/1 pair at wl=2), blocks of 289
# xpz: lower=z-even, upper=z-odd (di 0/1 pair at hj=wl=2), blocks of 289
U16 = (
    [("e", di * 578 + (hj % 2) * PE_ + (hj // 2) * 17, 2 * PLANE // 2, 128)
     for di in range(K) for hj in range(K)]
    + [("o", di * PE_ + 1, 578, 128) for di in range(K)]
    + [("z", 17 + 1, PE_, 128)]
    + [("o", 2 * PE_ + 17 + 1, 578, 64)]
)
NU16 = len(U16)  # 14

_PROGRAM_CACHE = {}


def _win(t, parts, offset, dims, p0=0):
    """Windowed AP view of a [P, L] tile: free dims [(stride, count), ...]."""
    v = t[p0 : p0 + parts, offset : offset + 1]
    for _ in range(len(dims) - 1):
        v = v.unsqueeze(1)
    w = v.copy()
    for i, (st, cnt) in enumerate(dims):
        w.ap[i + 1] = (st, cnt)
    return w


def _pack_w8(conv_w):
    """fp8 weight pack (128, 8*108): per unit u, col u*108 + kt*54 + m."""
    wq = (np.asarray(conv_w, np.float32) * 32.0).astype(E4).astype(np.float32) / 32.0
    # taps[(u, ktile, half)] = (di, hj, wl) or None
    tp = {}
    for u, di in enumerate(range(3)):  # U1-3
        tp[(u, 0, 0)] = (di, 0, 0); tp[(u, 0, 1)] = (di, 1, 0)
        tp[(u, 1, 0)] = (di, 0, 1); tp[(u, 1, 1)] = (di, 1, 1)
    tp[(3, 0, 0)] = (0, 0, 2); tp[(3, 0, 1)] = (0, 1, 2)
    tp[(3, 1, 0)] = (1, 0, 2); tp[(3, 1, 1)] = (1, 1, 2)
    tp[(4, 0, 0)] = (2, 0, 2); tp[(4, 0, 1)] = (2, 1, 2)
    tp[(5, 0, 0)] = (0, 2, 0); tp[(5, 0, 1)] = (1, 2, 0)
    tp[(5, 1, 0)] = (0, 2, 1); tp[(5, 1, 1)] = (1, 2, 1)
    tp[(6, 0, 0)] = (2, 2, 0)
    tp[(6, 1, 0)] = (2, 2, 1)
    tp[(7, 0, 0)] = (0, 2, 2); tp[(7, 0, 1)] = (1, 2, 2)
    tp[(7, 1, 0)] = (2, 2, 2)
    # k-tile stride must be a multiple of 16 (ISA s3_lw dual-fp8): pad to 64
    wpk = np.zeros((128, NU8 * 128), dtype=np.float32)
    for (u, kt, half), (di, hj, wl) in tp.items():
        col = u * 128 + kt * 64
        wpk[half * 64 : half * 64 + 64, col : col + 54] = 32.0 * wq[:, :, di, hj, wl].T
    return wpk.astype(E4)


def _pack_w16(conv_w):
    """fp16 strided-conv weight pack (128, 14*54)."""
    w = np.asarray(conv_w, np.float32)
    wpk = np.zeros((128, NU16 * COUT), dtype=np.float32)
    for u in range(9):
        di, hj = u // 3, u % 3
        wpk[0:64, u * COUT : (u + 1) * COUT] = w[:, :, di, hj, 0].T
        wpk[64:128, u * COUT : (u + 1) * COUT] = w[:, :, di, hj, 1].T
    for i, u in enumerate(range(9, 12)):
        wpk[0:64, u * COUT : (u + 1) * COUT] = w[:, :, i, 0, 2].T
        wpk[64:128, u * COUT : (u + 1) * COUT] = w[:, :, i, 1, 2].T
    wpk[0:64, 12 * COUT : 13 * COUT] = w[:, :, 0, 2, 2].T
    wpk[64:128, 12 * COUT : 13 * COUT] = w[:, :, 1, 2, 2].T
    wpk[0:64, 13 * COUT : 14 * COUT] = w[:, :, 2, 2, 2].T
    return wpk.astype(np.float16)


def _build_program_a():
    nc = bacc.Bacc(
        "TRN2", target_bir_lowering=False, debug=False, num_devices=N_CORES
    )
    x8a = nc.dram_tensor("x8a", (128, XB8A), F8, kind="ExternalInput").ap()
    x8b = nc.dram_tensor("x8b", (128, XB8B), F8, kind="ExternalInput").ap()
    wpk8 = nc.dram_tensor("wpk8", (128, NU8 * 128), F8, kind="ExternalInput").ap()
    xpe = nc.dram_tensor("xpe", (128, XPE_LEN), F16, kind="ExternalInput").ap()
    xpo = nc.dram_tensor("xpo", (128, XPO_LEN), F16, kind="ExternalInput").ap()
    xpz = nc.dram_tensor("xpz", (128, XPZ_LEN), F16, kind="ExternalInput").ap()
    wpk16 = nc.dram_tensor("wpk16", (128, NU16 * COUT), F16, kind="ExternalInput").ap()
    sq = nc.dram_tensor("sq", (COUT, CSPAN), F16, kind="ExternalOutput").ap()
    ssub16 = nc.dram_tensor("ssub16", (COUT, SPOS), F32, kind="ExternalOutput").ap()

    DR = mybir.MatmulPerfMode.DoubleRow

    with tile.TileContext(nc) as tc:
        with (
            tc.tile_pool(name="xin", bufs=1) as xin_pool,
            tc.tile_pool(name="consts", bufs=1) as const_pool,
            tc.tile_pool(name="sq", bufs=3) as sq_pool,
        ):
            W8 = const_pool.tile([128, NU8 * 128], F8)
            W16 = const_pool.tile([128, NU16 * COUT], F16)
            X8A = xin_pool.tile([128, XB8A], F8)
            X8B = xin_pool.tile([128, XB8B], F8)
            XPE = xin_pool.tile([128, XPE_LEN], F16)
            XPO = xin_pool.tile([128, XPO_LEN], F16)
            XPZ = xin_pool.tile([128, XPZ_LEN], F16)

            # weights first (needed by the first matmul), then fp8 slabs in
            # per-plane chunks split over the sync+scalar rings so the conv
            # starts early; parity tensors (needed last) trail on scalar
            nc.sync.dma_start(W8[:], wpk8[:])
            nc.scalar.dma_start(W16[:], wpk16[:])
            for z in range(ZP):
                lo = z * PLANE
                hia = XB8A if z == ZP - 1 else (z + 1) * PLANE
                hib = XB8B if z == ZP - 1 else (z + 1) * PLANE
                ra, rb = (nc.sync, nc.scalar) if z % 2 == 0 else (nc.scalar, nc.sync)
                ra.dma_start(X8A[:, lo:hia], x8a[:, lo:hia])
                rb.dma_start(X8B[:, lo:hib], x8b[:, lo:hib])
            nc.scalar.dma_start(XPE[:], xpe[:])
            nc.sync.dma_start(XPO[:], xpo[:])
            nc.scalar.dma_start(XPZ[:], xpz[:])

            # ---- fp8 stats conv: 8 DoubleRow units over the contiguous
            # 9248-col flat span (DoubleRow ifmap must be [K, 2, N]);
            # junk rows/cols are masked on the host
            SQ16 = sq_pool.tile([COUT, CSPAN], F16, bufs=1)
            with tc.tile_pool(name="psum_s", bufs=6, space="PSUM") as ps_pool:
                sq_done = 0
                for d in range(DL):
                    for c0, ns in [(0, 512), (512, 512), (1024, 64)]:
                        Pc = ps_pool.tile(
                            [COUT, 512], F32, tag="sps", name=f"S{d}_{c0}"
                        )
                        for u, (tn, basef, df) in enumerate(U8A):
                            xt = X8A if tn == "a" else X8B
                            lhsT = _win(W8, 128, u * 128, [(64, 2), (1, 54)])
                            rhs = _win(xt, 128, basef(d) + c0, [(df, 2), (1, ns)])
                            nc.tensor.matmul(
                                Pc[:, 0:ns], lhsT, rhs,
                                start=(u == 0), stop=(u == NU8 - 1),
                                perf_mode=DR,
                            )
                        nc.scalar.copy(
                            SQ16[:, d * SPAN + c0 : d * SPAN + c0 + ns], Pc[:, 0:ns]
                        )
                    if d % 2 == 1:
                        nc.gpsimd.dma_start(
                            sq[:, sq_done : (d + 1) * SPAN],
                            SQ16[:, sq_done : (d + 1) * SPAN],
                        )
                        sq_done = (d + 1) * SPAN

            # ---- fp16 strided conv: 14 units, 2 col chunks of 512
            with (
                tc.tile_pool(name="psum_t", bufs=2, space="PSUM") as pt_pool,
                tc.tile_pool(name="ssub", bufs=1) as ss_pool,
            ):
                Pt = [
                    pt_pool.tile([COUT, 512], F32, tag="spt", name=f"T{c}")
                    for c in range(2)
                ]
                for u, (tn, base, kst, ku) in enumerate(U16):
                    xt = {"e": XPE, "o": XPO, "z": XPZ}[tn]
                    lhsT = W16[0:ku, u * COUT : (u + 1) * COUT]
                    for c in range(2):
                        rhs = _win(
                            xt, ku, base + c * 2 * kst, [(kst, 2), (17, 16), (1, 16)]
                        )
                        nc.tensor.matmul(
                            Pt[c][:, 0:512], lhsT, rhs,
                            start=(u == 0), stop=(u == NU16 - 1),
                        )
                SS = ss_pool.tile([COUT, SPOS], F32)
                for c in range(2):
                    nc.scalar.copy(SS[:, c * 512 : (c + 1) * 512], Pt[c][:, 0:512])
                nc.gpsimd.dma_start(ssub16[:], SS[:])
    nc.compile()
    return nc


def _build_program_b():
    nc = bacc.Bacc(
        "TRN2", target_bir_lowering=False, debug=False, num_devices=N_CORES
    )
    # per pass P (chunk pair 2P, 2P+1): 3 di-planes x 4 parities x 289
    xq2 = nc.dram_tensor("xq2", (128, 2 * 3 * 4 * PE_), F16, kind="ExternalInput").ap()
    # en8 row r*4+s: attention for (group s%2, chunk 2P+s//2) at col
    # p*512 + P*256 + o; 3 replicas (r) live at partition bases 0/32/64
    # (matmul operand base-partition constraint) and spread the DMA
    en8 = nc.dram_tensor("en8", (12, 27 * 512), F16, kind="ExternalInput").ap()
    sel = nc.dram_tensor("sel", (12, 128), F16, kind="ExternalInput").ap()
    # row chunk*2+group, (27, 256) flat: source for the DMA partition
    # broadcasts that fill the late AREP blocks in parallel with the PE
    en = nc.dram_tensor("en", (8, 27 * 256), F16, kind="ExternalInput").ap()
    out = nc.dram_tensor("out", (64, SPOS), F16, kind="ExternalOutput").ap()

    OP = mybir.AluOpType
    QP = 3 * 4 * PE_  # 3468 cols per pass

    with tile.TileContext(nc) as tc:
        with (
            tc.tile_pool(name="xin", bufs=1) as xin_pool,
            tc.tile_pool(name="work", bufs=1) as work_pool,
            tc.tile_pool(name="psum_b", bufs=6, space="PSUM") as pb_pool,
        ):
            XQ2 = xin_pool.tile([128, 2 * QP], F16)
            EN8 = xin_pool.tile([68, 27 * 512], F16)
            SEL = xin_pool.tile([68, 128], F16)
            AREPS = [
                work_pool.tile([128, 27 * 256], F16, tag=f"arep{p}", name=f"AREP{p}")
                for p in range(2)
            ]
            BASES = [0, 32, 64]
            # compact attention + selector first, column-chunked so the
            # 27.6KB-per-partition-line writes spread across DMA engines
            EC = 27 * 512 // 2
            for r, pb in enumerate(BASES):
                nc.scalar.dma_start(SEL[pb : pb + 4, :], sel[4 * r : 4 * r + 4, :])
            for j in range(2):
                for r, pb in enumerate(BASES):
                    nc.scalar.dma_start(
                        EN8[pb : pb + 4, j * EC : (j + 1) * EC],
                        en8[4 * r : 4 * r + 4, j * EC : (j + 1) * EC],
                    )
            # slab on the sync ring (pass 0 first)
            nc.sync.dma_start(XQ2[:, 0:QP], xq2[:, 0:QP])
            nc.sync.dma_start(XQ2[:, QP : 2 * QP], xq2[:, QP : 2 * QP])
            # replicate attention across channel partitions on the (idle)
            # tensor engine: out[c, (q, o)] = en8[4r + s(c), ...], then ACT
            # converts PSUM fp32 -> AREP fp16
            for p in range(2):
                for b in range(5):  # tap-pair blocks q = 2b, 2b+1 (b=4: q=8)
                    nq = 2 if b < 4 else 1
                    for di in range(3):
                        pb = BASES[(p * 15 + b * 3 + di) % 3]
                        rhs = _win(
                            EN8, 4,
                            (di * 9 + 2 * b) * 512 + p * 256,
                            [(512, nq), (1, 256)], p0=pb,
                        )
                        PB = pb_pool.tile(
                            [128, 512], F32, tag="bb", name=f"B{p}_{b}_{di}"
                        )
                        nc.tensor.matmul(
                            PB[:, 0 : nq * 256], SEL[pb : pb + 4, 0:128], rhs,
                            start=True, stop=True,
                        )
                        dst = _win(
                            AREPS[p], 128, (di * 9 + 2 * b) * 256,
                            [(256, nq), (1, 256)],
                        )
                        src = _win(PB, 128, 0, [(256, nq), (1, 256)])
                        nc.scalar.copy(dst, src)
            for p in range(2):
                PRD = work_pool.tile([128, 27 * 256], F16, tag="prd", bufs=2)
                for hj in range(K):
                    for wl in range(K):
                        xb = (
                            p * QP + (hj % 2) * 578 + (wl % 2) * PE_
                            + (hj // 2) * 17 + (wl // 2)
                        )
                        xv = _win(XQ2, 128, xb, [(4 * PE_, 3), (17, 16), (1, 16)])
                        p0 = (hj * K + wl) * 256
                        av = _win(AREPS[p], 128, p0, [(9 * 256, 3), (16, 16), (1, 16)])
                        pv = _win(PRD, 128, p0, [(9 * 256, 3), (16, 16), (1, 16)])
                        nc.vector.tensor_tensor(pv, xv, av, op=OP.mult)
                # halving tree over the 27 tap blocks (fp16, final add fp32)
                for dst, src0, src1 in [
                    (13, 14, 27), (7, 7, 14), (3, 4, 7), (2, 2, 4),
                ]:
                    nc.vector.tensor_add(
                        PRD[:, 0 : dst * 256],
                        PRD[:, 0 : dst * 256],
                        PRD[:, src0 * 256 : src1 * 256],
                    )
                OUTB = work_pool.tile([128, 256], F16, tag="outb", bufs=2)
                nc.vector.tensor_add(OUTB[:], PRD[:, 0:256], PRD[:, 256:512])
                nc.gpsimd.dma_start(
                    out[:, (2 * p) * 256 : (2 * p + 1) * 256], OUTB[0:64, :]
                )
                nc.gpsimd.dma_start(
                    out[:, (2 * p + 1) * 256 : (2 * p + 2) * 256], OUTB[64:128, :]
                )
    nc.compile()
    return nc


def _prep_inputs(x, conv_w):
    xpad = np.pad(
        np.asarray(x, dtype=np.float32),
        ((0, 0), (0, 0), (1, 1), (1, 1), (1, 1)),
        mode="reflect",
    )
    xq8 = xpad.astype(E4)
    x16 = xpad.astype(np.float16)
    w8 = _pack_w8(conv_w)
    w16 = _pack_w16(conv_w)
    in_a, in_b_x = [], []
    for core in range(N_CORES):
        n, dc = core // 4, core % 4
        s8 = xq8[n, :, 8 * dc : 8 * dc + ZP].reshape(C, XLEN)
        x8a = np.zeros((128, XB8A), dtype=E4)
        x8a[0:64, :XLEN] = s8
        x8a[64:128, : XLEN - 34] = s8[:, 34:]
        x8b = np.zeros((128, XB8B), dtype=E4)
        x8b[0:64, :XLEN] = s8
        x8b[64:128, : XLEN - PLANE] = s8[:, PLANE:]
        s16 = x16[n, :, 8 * dc : 8 * dc + ZP]  # (C, 10, 34, 34)
        # parity volumes P[z, py, px] = s16[:, z, py::2, px::2] (17, 17)
        pv = np.zeros((C, ZP, 2, 2, 17, 17), dtype=np.float16)
        for py in range(2):
            for px in range(2):
                pv[:, :, py, px] = s16[:, :, py::2, px::2]
        xpe = np.zeros((128, XPE_LEN), dtype=np.float16)
        xpe[0:64] = pv[:, 0:9, :, 0].reshape(C, 9 * 2 * PE_)
        xpe[64:128] = pv[:, 0:9, :, 1].reshape(C, 9 * 2 * PE_)
        xpo = np.zeros((128, XPO_LEN), dtype=np.float16)
        xpo[0:64] = pv[:, 0:9, 0, 0].reshape(C, 9 * PE_)
        xpo[64:128] = pv[:, 0:9, 1, 0].reshape(C, 9 * PE_)
        xpz = np.zeros((128, XPZ_LEN), dtype=np.float16)
        xpz[0:64] = pv[:, 0:10:2, 0, 0].reshape(C, 5 * PE_)
        xpz[64:128] = pv[:, 1:10:2, 0, 0].reshape(C, 5 * PE_)
        in_a.append({
            "x8a": x8a, "x8b": x8b, "wpk8": w8,
            "xpe": xpe, "xpo": xpo, "xpz": xpz, "wpk16": w16,
        })
        # launch B slab: per pass P, chunk pair (2P, 2P+1):
        # col = P*3468 + di*1156 + py*578 + px*289 + r*17 + q
        xq2 = np.zeros((128, 2 * 3 * 4 * PE_), dtype=np.float16)
        for p in range(2):
            for half in range(2):
                zb = 4 * p + 2 * half  # chunk k = 2p+half, planes zb..zb+2
                blk = pv[:, zb : zb + 3].reshape(C, 3 * 4 * PE_)
                xq2[half * 64 : half * 64 + 64,
                    p * 3 * 4 * PE_ : (p + 1) * 3 * 4 * PE_] = blk
        in_b_x.append(xq2)
    return in_a, in_b_x


def kernel(x, conv_w, bn_gamma, bn_beta):
    if "a" not in _PROGRAM_CACHE:
        _PROGRAM_CACHE["a"] = _build_program_a()
        _PROGRAM_CACHE["b"] = _build_program_b()
    nca, ncb = _PROGRAM_CACHE["a"], _PROGRAM_CACHE["b"]

    in_a, in_b_x = _prep_inputs(x, conv_w)
    res_a = bass_utils.run_bass_kernel_spmd(nca, in_a, core_ids=list(range(N_CORES)))

    # ---- host: BN stats from the fp8 conv (bias-corrected), then softmax
    S = np.zeros(COUT); S2 = np.zeros(COUT)
    m8 = np.zeros(COUT); v8 = np.zeros(COUT)
    m16 = np.zeros(COUT); v16 = np.zeros(COUT)
    subs16 = []
    for core in range(N_CORES):
        sqv = res_a.results[core]["sq"].reshape(COUT, DL, H, PW)[:, :, :, :W]
        sqv = sqv.astype(np.float64) / 32.0  # wpk8 is pre-scaled by 32
        S += sqv.sum(axis=(1, 2, 3)); S2 += (sqv * sqv).sum(axis=(1, 2, 3))
        s8 = sqv[:, ::2, ::2, ::2].reshape(COUT, SPOS)
        m8 += s8.sum(axis=1); v8 += (s8 * s8).sum(axis=1)
        s16 = res_a.results[core]["ssub16"].astype(np.float64)
        m16 += s16.sum(axis=1); v16 += (s16 * s16).sum(axis=1)
        subs16.append(s16)
    n_s = float(N_CORES * SPOS)
    mean_q, var_q = S / M_TOTAL, S2 / M_TOTAL - (S / M_TOTAL) ** 2
    m8, m16 = m8 / n_s, m16 / n_s
    v8, v16 = v8 / n_s - m8 * m8, v16 / n_s - m16 * m16
    mean_c = mean_q - (m8 - m16)
    var_c = var_q - (v8 - v16)
    rstd = 1.0 / np.sqrt(var_c + EPS)
    a = np.asarray(bn_gamma, np.float64) * rstd
    b = np.asarray(bn_beta, np.float64) - mean_c * a

    # selector weights: SEL[s, c] = 1 iff chunk-half(c) == s//2 and group(c) == s%2
    selw = np.zeros((4, 128), dtype=np.float16)
    for s in range(4):
        for c in range(128):
            if c // 64 == s // 2 and (c % 64) // 32 == s % 2:
                selw[s, c] = 1.0
    selw = np.tile(selw, (3, 1))  # 3 replicas at partition bases 0/32/64
    in_b = []
    for core in range(N_CORES):
        z = a[:, None] * subs16[core] + b[:, None]
        e = np.exp(z - z.max(axis=0, keepdims=True))
        en = (e / e.sum(axis=0, keepdims=True))
        # en8[r*4+s, p*512 + P*256 + o] = attn[group s%2, tap p, chunk 2P+s//2, o]
        env = en.reshape(GROUP, 27, 4, 256)  # (g, p, chunk, o)
        e8 = np.zeros((4, 27, 2, 256), dtype=np.float16)  # (s, p, P, o)
        for s in range(4):
            g, half = s % 2, s // 2
            e8[s] = env[g, :, half::2, :]  # chunks (half, half+2) -> P in {0,1}
        e8 = e8.reshape(1, 4, 27 * 512)
        en8 = np.broadcast_to(e8, (3, 4, 27 * 512)).reshape(12, 27 * 512)
        enr = np.ascontiguousarray(
            en.reshape(GROUP, 27, 4, 256).transpose(2, 0, 1, 3)
        ).reshape(8, 27 * 256).astype(np.float16)
        in_b.append({
            "xq2": in_b_x[core],
            "en8": np.ascontiguousarray(en8),
            "sel": selw,
            "en": enr,
        })
    res_b = bass_utils.run_bass_kernel_spmd(ncb, in_b, core_ids=list(range(N_CORES)))

    full = np.empty((N, C, D // 2, H // 2, W // 2), dtype=np.float32)
    for core in range(N_CORES):
        n, dc = core // 4, core % 4
        full[n, :, 4 * dc : 4 * dc + 4] = res_b.results[core]["out"].astype(
            np.float32
        ).reshape(64, 4, 16, 16)
    return full
